# revision 21
# baseline (speedup 1.0000x reference)
"""Trainium2 Bass kernel for nn_CriticNetwork (gnn_message_passing).

Key mathematical simplification (verified numerically against the
reference): the reference broadcasts edge_index to (B, 2, E) and
reshapes to (2, B*E).  Row-major reshape interleaves the src/dst
blocks so the resulting src and dst arrays are ELEMENTWISE EQUAL --
every edge is a self-edge v->v.  With GCN normalization
(deg = 1 + 2*count(v), each self-edge contributes x[v]/deg, plus the
explicit self-loop) the aggregate is exactly deg * x[v]/deg = x[v].
Both GCNConv layers therefore collapse to plain linear layers:

    x = relu(x @ W1 + b1); x = relu(x @ W2 + b2)
    node_avg[b] = mean_n(x[b, n] @ node_fc_W) + node_fc_b
    col path is a plain 2-layer MLP; final head is a tiny [4,2] MLP.

Since node_fc / col_W2 are applied linearly after the last relu, the
device only needs per-(batch-slice) SUMS of the hidden activations:
each core processes 25000 nodes (half a batch) + 500 col rows and
returns two small accumulator vectors; the host applies the final
(tiny) linear head.

Device layout per core:
  xT_packed [128, 12500]: rows 0-63  = 64 features of nodes [0, 12500)
                          rows 64-127 = 64 features of nodes [12500, 25000)
  L1 matmul: lhsT = blockdiag(W1, W1) [128, 32] -> h1.T bands [32, 512]
  4 L1 matmuls stack bands in one PSUM bank -> [128, 512]
  relu (ScalarE, bias fused) -> SBUF
  L2 matmul: lhsT = blockdiag(W2 x8) [128, 128] -> [128, 512] PSUM
  relu + accumulate (ScalarE accum_out = per-partition row sum)
  final: reduce accum columns -> node_acc [128, 1] (8 bands of 16)

All constants (weights, biases, col features) ship in ONE packed DMA
("wpack") and a zero-valued warmup matmul consumes it first: the PE
LDWEIGHTS instruction can carry only ONE semaphore wait, so every real
matmul must depend on at most one un-synced DMA lane (its x chunk).
"""

import numpy as np

import concourse.bacc as bacc
import concourse.bass as bass
import concourse.mybir as mybir
import concourse.tile as tile
from concourse.bass_utils import run_bass_kernel_spmd

P = 128
N_CORES = 8
B, N, F_NODE, H = 4, 50000, 64, 16
NODES_PER_CORE = (B * N) // N_CORES        # 25000
COLS = NODES_PER_CORE // 2                 # 12500 packed columns (2 nodes/col)
MM = 512                                   # fp32 matmul max moving free dim
SUPER = 4 * MM                             # 2048 columns per PSUM-bank group
N_CHUNKS = (COLS + SUPER - 1) // SUPER     # 7 (6 full + 212-col tail)
C, F_COL = 1000, 32
COLN = (B * C) // N_CORES                  # 500 col rows per core

# wpack column layout
W1_OFF = 0                                  # [128, 32] blockdiag(W1, W1)
W2_OFF = W1_OFF + 2 * H                     # [128, 128] blockdiag(W2 x8)
B1_OFF = W2_OFF + P                         # [128, 1] b1 tiled x8
B2_OFF = B1_OFF + 1                         # [128, 1] b2 tiled x8
CW1_OFF = B2_OFF + 1                        # [32, 16] col_W1 (rows 0-31)
CB1_OFF = CW1_OFF + H                       # [16, 1] col_b1 (rows 0-15)
ZPAD_OFF = CB1_OFF + 1                      # [128, 1] zeros (warmup operand)
COLT_OFF = ZPAD_OFF + 1                     # [32, 500] colT (rows 0-31)
NW = COLT_OFF + COLN                        # 680

DT = mybir.dt.float32                      # compute dtype on device
NPDT = np.float32

PROFILE = False        # set True (e.g. from test.py) to collect NTFF timing
CHECK_WAITS = True     # build-time guard: one semaphore wait per compute inst
LAST_EXEC_TIME_NS = None

_NC_CACHE = {}


def _build_nc():
    f32 = mybir.dt.float32
    Relu = mybir.ActivationFunctionType.Relu
    # Bacc (not raw Bass): its finalize() runs move_matmul_waits_to_-
    # ldweights + generate_event_semaphores, which legalize schedules
    # against the TRN2 one-semaphore-wait-per-instruction limit.
    nc = bacc.Bacc("TRN2")

    xT = nc.dram_tensor("xT", [P, COLS], DT, kind="ExternalInput")
    wpack = nc.dram_tensor("wpack", [P, NW], DT, kind="ExternalInput")
    node_acc = nc.dram_tensor("node_acc", [P, 1], f32, kind="ExternalOutput")
    col_acc = nc.dram_tensor("col_acc", [H, 1], f32, kind="ExternalOutput")

    with tile.TileContext(nc) as tc:
        with (
            tc.tile_pool(name="consts", bufs=1) as consts,
            tc.tile_pool(name="xin", bufs=3) as xin,
            tc.tile_pool(name="work", bufs=2) as work,
            tc.tile_pool(name="outp", bufs=1) as outp,
            tc.tile_pool(name="psum", bufs=1, space="PSUM") as psum,
        ):
            wp = consts.tile([P, NW], DT)
            nc.sync.dma_start(wp[:], wpack[:])
            w1_t = wp[:, W1_OFF:W1_OFF + 2 * H]
            w2_t = wp[:, W2_OFF:W2_OFF + P]
            b1_t = wp[:, B1_OFF:B1_OFF + 1]
            b2_t = wp[:, B2_OFF:B2_OFF + 1]
            cw1_t = wp[:F_COL, CW1_OFF:CW1_OFF + H]
            cb1_t = wp[:H, CB1_OFF:CB1_OFF + 1]
            zc_t = wp[:, ZPAD_OFF:ZPAD_OFF + 1]
            colT_t = wp[:F_COL, COLT_OFF:COLT_OFF + COLN]

            # Zero stats ON the scalar engine (a gpsimd memset would add a
            # Pool-sem wait to the first ACT accumulate -- TRN2 compute
            # instructions carry at most ONE semaphore wait).  Reading
            # wpack here also syncs ACT with the wpack DMA lane up front.
            stats = outp.tile([P, N_CHUNKS + 1], f32)
            nc.scalar.mul(stats[:], wp[:, :N_CHUNKS + 1], 0.0)

            # Persistent PSUM tiles (allocated once, manually alternated):
            # a per-chunk pool tile would get a slot-recycle writer guard,
            # an extra PE-sem wait on the first matmul of each chunk -- and
            # the PE LDWEIGHTS instruction can carry only ONE wait.
            ps1_t = [psum.tile([P, MM], f32, tag=f"ps1_{k}", name=f"ps1_{k}")
                     for k in range(2)]
            ps2_t = [psum.tile([P, MM], f32, tag=f"ps2_{k}", name=f"ps2_{k}")
                     for k in range(2)]
            h1r_t = [work.tile([P, MM], f32, tag=f"h1r_{k}", name=f"h1r_{k}")
                     for k in range(2)]
            scr_t = [work.tile([P, MM], f32, tag=f"scr_{k}", name=f"scr_{k}")
                     for k in range(2)]

            # Warmup matmul: syncs PE with the wpack DMA using a single
            # wait, so every later matmul has the wpack lane subsumed.
            # Reads the zero pad column -> contributes exactly 0.0 to
            # stats' spare column (kept live through that write).
            psd = psum.tile([1, 1], f32, tag="psd")
            nc.tensor.matmul(psd[0:1, 0:1], zc_t, zc_t, start=True, stop=True)
            nc.scalar.copy(stats[0:1, N_CHUNKS:N_CHUNKS + 1], psd[0:1, 0:1])

            for s in range(N_CHUNKS):
                c0 = s * SUPER
                cols = min(SUPER, COLS - c0)
                nb = (cols + MM - 1) // MM
                act_w = cols if nb == 1 else cols // nb
                assert act_w * nb == cols, (s, cols, nb)

                x_t = xin.tile([P, SUPER], DT, tag="x")
                nc.sync.dma_start(x_t[:, :cols], xT[:, c0:c0 + cols])

                ps1 = ps1_t[s % 2]
                for bnd in range(nb):
                    w = min(MM, cols - bnd * MM)
                    nc.tensor.matmul(
                        ps1[32 * bnd:32 * bnd + 32, :w],
                        w1_t,
                        x_t[:, bnd * MM:bnd * MM + w],
                        start=True, stop=True,
                        tile_position=(0, 32 * bnd),
                    )
                used = 32 * nb

                h1r = h1r_t[s % 2]
                nc.scalar.activation(
                    h1r[:used, :act_w], ps1[:used, :act_w], Relu,
                    bias=b1_t[:used, :],
                )

                ps2 = ps2_t[s % 2]
                nc.tensor.matmul(
                    ps2[:used, :act_w],
                    w2_t[:used, :used],
                    h1r[:used, :act_w],
                    start=True, stop=True,
                )
                scr = scr_t[s % 2]
                nc.scalar.activation(
                    scr[:used, :act_w], ps2[:used, :act_w], Relu,
                    bias=b2_t[:used, :],
                    accum_out=stats[:used, s:s + 1],
                )

            # column-features path (tiny): h = relu(col @ col_W1 + col_b1)
            psc = psum.tile([H, COLN], f32, tag="psc")
            nc.tensor.matmul(psc[:, :], cw1_t, colT_t, start=True, stop=True)
            colscr = outp.tile([H, COLN], f32)
            col_sb = outp.tile([H, 1], f32)
            nc.scalar.activation(colscr[:], psc[:], Relu,
                                 bias=cb1_t, accum_out=col_sb[:])

            node_sb = outp.tile([P, 1], f32)
            nc.vector.tensor_reduce(node_sb[:], stats[:],
                                    axis=mybir.AxisListType.X,
                                    op=mybir.AluOpType.add)
            nc.sync.dma_start(node_acc[:], node_sb[:])
            nc.sync.dma_start(col_acc[:], col_sb[:])

    nc.finalize()

    # Verify the legalization: at most one wait per instruction
    # (InstEventSemaphore may carry two).
    if CHECK_WAITS:
        for blk in nc.m.functions[0].blocks:
            for inst in blk.instructions:
                si = inst.sync_info
                nwait = len(si.on_wait) if si and si.on_wait else 0
                limit = 2 if type(inst).__name__ in (
                    "InstEventSemaphore", "InstDrain", "InstDMACopy") else 1
                assert nwait <= limit, (
                    inst.name, type(inst).__name__,
                    [w.ant_name for w in si.on_wait])
    return nc


def _get_nc():
    if "nc" not in _NC_CACHE:
        _NC_CACHE["nc"] = _build_nc()
    return _NC_CACHE["nc"]


def _prep_in_maps(node_features, col_features, W1, b1, W2, b2, col_W1, col_b1):
    x = np.ascontiguousarray(node_features, dtype=np.float32).reshape(B * N, F_NODE)
    colf = np.ascontiguousarray(col_features, dtype=np.float32).reshape(B * C, F_COL)

    W1 = np.asarray(W1, np.float32)
    W2 = np.asarray(W2, np.float32)
    wpack = np.zeros((P, NW), np.float32)
    wpack[:F_NODE, W1_OFF:W1_OFF + H] = W1
    wpack[F_NODE:, W1_OFF + H:W1_OFF + 2 * H] = W1
    for i in range(P // H):
        wpack[H * i:H * i + H, W2_OFF + H * i:W2_OFF + H * i + H] = W2
    wpack[:, B1_OFF] = np.tile(np.asarray(b1, np.float32), P // H)
    wpack[:, B2_OFF] = np.tile(np.asarray(b2, np.float32), P // H)
    wpack[:F_COL, CW1_OFF:CW1_OFF + H] = np.asarray(col_W1, np.float32)
    wpack[:H, CB1_OFF] = np.asarray(col_b1, np.float32)

    in_maps = []
    for c in range(N_CORES):
        n0 = c * NODES_PER_CORE
        half = NODES_PER_CORE // 2
        xa = x[n0:n0 + half].T                      # [64, 12500] view
        xb = x[n0 + half:n0 + NODES_PER_CORE].T
        xT = np.ascontiguousarray(
            np.concatenate([xa, xb], axis=0), dtype=np.float32).astype(NPDT)
        wp = wpack.copy()
        wp[:F_COL, COLT_OFF:COLT_OFF + COLN] = colf[c * COLN:(c + 1) * COLN].T
        in_maps.append({"xT": xT, "wpack": wp.astype(NPDT)})
    return in_maps


def kernel(node_features, col_features, edge_index, W1, b1, W2, b2,
           node_fc_W, node_fc_b, col_W1, col_b1, col_W2, col_b2,
           fc_W, fc_b, out_W, out_b):
    global LAST_EXEC_TIME_NS
    # edge_index provably does not affect the output (see module docstring).
    in_maps = _prep_in_maps(node_features, col_features,
                            W1, b1, W2, b2, col_W1, col_b1)
    nc = _get_nc()
    res = run_bass_kernel_spmd(nc, in_maps, core_ids=list(range(N_CORES)),
                               trace=PROFILE)
    LAST_EXEC_TIME_NS = res.exec_time_ns
    outs = res.results

    node_fc_W = np.asarray(node_fc_W, np.float32)
    col_W2 = np.asarray(col_W2, np.float32)
    node_avg = np.zeros((B, 1), np.float32)
    col_avg = np.zeros((B, 1), np.float32)
    for b in range(B):
        ns = (outs[2 * b]["node_acc"].reshape(P // H, H).sum(axis=0) +
              outs[2 * b + 1]["node_acc"].reshape(P // H, H).sum(axis=0))
        cs = (outs[2 * b]["col_acc"].reshape(H) +
              outs[2 * b + 1]["col_acc"].reshape(H))
        node_avg[b, 0] = (ns / np.float32(N)) @ node_fc_W[:, 0] + \
            np.asarray(node_fc_b, np.float32)[0]
        col_avg[b, 0] = (cs / np.float32(C)) @ col_W2[:, 0] + \
            np.asarray(col_b2, np.float32)[0]

    combined = np.concatenate([node_avg, col_avg], axis=1)      # [B, 2]
    z = np.maximum(combined @ np.asarray(fc_W, np.float32) +
                   np.asarray(fc_b, np.float32), 0.0)
    out = z @ np.asarray(out_W, np.float32) + np.asarray(out_b, np.float32)
    return out.astype(np.float32)


# revision 30
# speedup vs baseline: 1.4377x; 1.4377x over previous
"""Trainium2 Bass kernel for nn_CriticNetwork (gnn_message_passing).

Key mathematical simplification (verified numerically against the
reference): the reference broadcasts edge_index to (B, 2, E) and
reshapes to (2, B*E).  Row-major reshape interleaves the src/dst
blocks so the resulting src and dst arrays are ELEMENTWISE EQUAL --
every edge is a self-edge v->v.  With GCN normalization
(deg = 1 + 2*count(v), each self-edge contributes x[v]/deg, plus the
explicit self-loop) the aggregate is exactly deg * x[v]/deg = x[v].
Both GCNConv layers therefore collapse to plain linear layers:

    x = relu(x @ W1 + b1); x = relu(x @ W2 + b2)
    node_avg[b] = mean_n(x[b, n] @ node_fc_W) + node_fc_b
    col path is a plain 2-layer MLP; final head is a tiny [4,2] MLP.

Since node_fc / col_W2 are applied linearly after the last relu, the
device only needs per-(batch-slice) SUMS of the hidden activations:
each core processes 25000 nodes (half a batch) + 500 col rows and
returns two small accumulator vectors; the host applies the final
(tiny) linear head.

Device layout per core:
  xT_packed [128, 12500]: rows 0-63  = 64 features of nodes [0, 12500)
                          rows 64-127 = 64 features of nodes [12500, 25000)
  L1 matmul: lhsT = blockdiag(W1, W1) [128, 32] -> h1.T bands [32, 512]
  4 L1 matmuls stack bands in one PSUM bank -> [128, 512]
  relu (ScalarE, bias fused) -> SBUF
  L2 matmul: lhsT = blockdiag(W2 x8) [128, 128] -> [128, 512] PSUM
  relu + accumulate (ScalarE accum_out = per-partition row sum)
  final: reduce accum columns -> node_acc [128, 1] (8 bands of 16)

All constants (weights, biases, col features) ship in ONE packed DMA
("wpack") and a zero-valued warmup matmul consumes it first: the PE
LDWEIGHTS instruction can carry only ONE semaphore wait, so every real
matmul must depend on at most one un-synced DMA lane (its x chunk).
"""

import ml_dtypes
import numpy as np

import concourse.bacc as bacc
import concourse.bass as bass
import concourse.mybir as mybir
import concourse.tile as tile
from concourse.bass_utils import run_bass_kernel_spmd

P = 128
N_CORES = 8
B, N, F_NODE, H = 4, 50000, 64, 16
NODES_PER_CORE = (B * N) // N_CORES        # 25000
COLS = NODES_PER_CORE // 2                 # 12500 packed columns (2 nodes/col)
MM = 512                                   # fp32 matmul max moving free dim
SUPER = 4 * MM                             # 2048 columns per PSUM-bank group
N_CHUNKS = (COLS + SUPER - 1) // SUPER     # 7 (6 full + 212-col tail)
C, F_COL = 1000, 32
COLN = (B * C) // N_CORES                  # 500 col rows per core

# wpack column layout
W1_OFF = 0                                  # [128, 32] blockdiag(W1, W1)
W2_OFF = W1_OFF + 2 * H                     # [128, 128] blockdiag(W2 x8)
B1_OFF = W2_OFF + P                         # [128, 1] b1 tiled x8
B2_OFF = B1_OFF + 1                         # [128, 1] b2 tiled x8
CW1_OFF = B2_OFF + 1                        # [32, 16] col_W1 (rows 0-31)
CB1_OFF = CW1_OFF + H                       # [16, 1] col_b1 (rows 0-15)
ZPAD_OFF = CB1_OFF + 1                      # [128, 1] zeros (warmup operand)
COLT_OFF = ZPAD_OFF + 1                     # [32, 500] colT (rows 0-31)
NW = COLT_OFF + COLN                        # 680

DT = mybir.dt.bfloat16                     # matmul-operand dtype on device
NPDT = ml_dtypes.bfloat16

PROFILE = False        # set True (e.g. from test.py) to collect NTFF timing
CHECK_WAITS = True     # build-time guard: one semaphore wait per compute inst
LAST_EXEC_TIME_NS = None
LAST_RESULTS = None

_NC_CACHE = {}


def _build_nc(relu1_on_dve=True):
    f32 = mybir.dt.float32
    Relu = mybir.ActivationFunctionType.Relu
    # Bacc (not raw Bass): its finalize() runs move_matmul_waits_to_-
    # ldweights + generate_event_semaphores, which legalize schedules
    # against the TRN2 one-semaphore-wait-per-instruction limit.
    nc = bacc.Bacc("TRN2")

    xT = nc.dram_tensor("xT", [P, COLS], DT, kind="ExternalInput")
    wpack = nc.dram_tensor("wpack", [P, NW], DT, kind="ExternalInput")
    node_acc = nc.dram_tensor("node_acc", [P, 1], f32, kind="ExternalOutput")
    col_acc = nc.dram_tensor("col_acc", [H, 1], f32, kind="ExternalOutput")

    with tile.TileContext(nc) as tc:
        with (
            tc.tile_pool(name="consts", bufs=1) as consts,
            tc.tile_pool(name="xin", bufs=3) as xin,
            tc.tile_pool(name="work", bufs=2) as work,
            tc.tile_pool(name="outp", bufs=1) as outp,
            tc.tile_pool(name="psum", bufs=1, space="PSUM") as psum,
        ):
            wp = consts.tile([P, NW], DT)
            nc.sync.dma_start(wp[:], wpack[:])
            w1_t = wp[:, W1_OFF:W1_OFF + 2 * H]
            w2_t = wp[:, W2_OFF:W2_OFF + P]
            b1_t = wp[:, B1_OFF:B1_OFF + 1]
            b2_t = wp[:, B2_OFF:B2_OFF + 1]
            cw1_t = wp[:F_COL, CW1_OFF:CW1_OFF + H]
            cb1_t = wp[:H, CB1_OFF:CB1_OFF + 1]
            zc_t = wp[:, ZPAD_OFF:ZPAD_OFF + 1]
            colT_t = wp[:F_COL, COLT_OFF:COLT_OFF + COLN]

            # Zero stats ON the scalar engine (a gpsimd memset would add a
            # Pool-sem wait to the first ACT accumulate -- TRN2 compute
            # instructions carry at most ONE semaphore wait).  Reading
            # wpack here also syncs ACT with the wpack DMA lane up front.
            stats = outp.tile([P, N_CHUNKS + 1], f32)
            nc.scalar.mul(stats[:], wp[:, :N_CHUNKS + 1], 0.0)

            # Persistent PSUM tiles (allocated once, manually alternated):
            # a per-chunk pool tile would get a slot-recycle writer guard,
            # an extra PE-sem wait on the first matmul of each chunk -- and
            # the PE LDWEIGHTS instruction can carry only ONE wait.
            ps1_t = [psum.tile([P, MM], f32, tag=f"ps1_{k}", name=f"ps1_{k}")
                     for k in range(2)]
            ps2_t = [psum.tile([P, MM], f32, tag=f"ps2_{k}", name=f"ps2_{k}")
                     for k in range(2)]
            h1r_t = [work.tile([P, MM], DT, tag=f"h1r_{k}", name=f"h1r_{k}")
                     for k in range(2)]
            scr_t = [work.tile([P, MM], DT, tag=f"scr_{k}", name=f"scr_{k}")
                     for k in range(2)]

            # Warmup matmul: syncs PE with the wpack DMA using a single
            # wait, so every later matmul has the wpack lane subsumed.
            # Reads the zero pad column -> contributes exactly 0.0 to
            # stats' spare column (kept live through that write).
            psd = psum.tile([1, 1], f32, tag="psd")
            nc.tensor.matmul(psd[0:1, 0:1], zc_t, zc_t, start=True, stop=True)
            nc.scalar.copy(stats[0:1, N_CHUNKS:N_CHUNKS + 1], psd[0:1, 0:1])

            for s in range(N_CHUNKS):
                c0 = s * SUPER
                cols = min(SUPER, COLS - c0)
                nb = (cols + MM - 1) // MM
                act_w = cols if nb == 1 else cols // nb
                assert act_w * nb == cols, (s, cols, nb)

                x_t = xin.tile([P, SUPER], DT, tag="x")
                nc.sync.dma_start(x_t[:, :cols], xT[:, c0:c0 + cols])

                ps1 = ps1_t[s % 2]
                for bnd in range(nb):
                    w = min(MM, cols - bnd * MM)
                    nc.tensor.matmul(
                        ps1[32 * bnd:32 * bnd + 32, :w],
                        w1_t,
                        x_t[:, bnd * MM:bnd * MM + w],
                        start=True, stop=True,
                        tile_position=(0, 32 * bnd),
                    )
                used = 32 * nb

                h1r = h1r_t[s % 2]
                if relu1_on_dve:
                    # b1 is structurally zero (setup_inputs uses
                    # jnp.zeros), so relu1 is a plain max with an
                    # immediate -- keeps DVE free of a wpack-DMA wait.
                    nc.vector.tensor_scalar_max(
                        h1r[:used, :act_w], ps1[:used, :act_w], 0.0)
                else:
                    nc.scalar.activation(
                        h1r[:used, :act_w], ps1[:used, :act_w], Relu,
                        bias=b1_t[:used, :],
                    )

                ps2 = ps2_t[s % 2]
                nc.tensor.matmul(
                    ps2[:used, :act_w],
                    w2_t[:used, :used],
                    h1r[:used, :act_w],
                    start=True, stop=True,
                )
                scr = scr_t[s % 2]
                nc.scalar.activation(
                    scr[:used, :act_w], ps2[:used, :act_w], Relu,
                    bias=b2_t[:used, :],
                    accum_out=stats[:used, s:s + 1],
                )

            # column-features path (tiny): h = relu(col @ col_W1 + col_b1)
            psc = psum.tile([H, COLN], f32, tag="psc")
            nc.tensor.matmul(psc[:, :], cw1_t, colT_t, start=True, stop=True)
            colscr = outp.tile([H, COLN], f32)
            col_sb = outp.tile([H, 1], f32)
            nc.scalar.activation(colscr[:], psc[:], Relu,
                                 bias=cb1_t, accum_out=col_sb[:])

            node_sb = outp.tile([P, 1], f32)
            nc.vector.tensor_reduce(node_sb[:], stats[:],
                                    axis=mybir.AxisListType.X,
                                    op=mybir.AluOpType.add)
            nc.sync.dma_start(node_acc[:], node_sb[:])
            nc.sync.dma_start(col_acc[:], col_sb[:])

    nc.finalize()

    # Verify the legalization: at most one wait per instruction
    # (InstEventSemaphore may carry two).
    if CHECK_WAITS:
        for blk in nc.m.functions[0].blocks:
            for inst in blk.instructions:
                si = inst.sync_info
                nwait = len(si.on_wait) if si and si.on_wait else 0
                limit = 2 if type(inst).__name__ in (
                    "InstEventSemaphore", "InstDrain", "InstDMACopy") else 1
                assert nwait <= limit, (
                    inst.name, type(inst).__name__,
                    [w.ant_name for w in si.on_wait])
    return nc


def _get_nc(relu1_on_dve=True):
    key = ("nc", relu1_on_dve)
    if key not in _NC_CACHE:
        _NC_CACHE[key] = _build_nc(relu1_on_dve)
    return _NC_CACHE[key]


def _prep_in_maps(node_features, col_features, W1, b1, W2, b2, col_W1, col_b1):
    x = np.ascontiguousarray(node_features, dtype=np.float32).reshape(B * N, F_NODE)
    colf = np.ascontiguousarray(col_features, dtype=np.float32).reshape(B * C, F_COL)

    W1 = np.asarray(W1, np.float32)
    W2 = np.asarray(W2, np.float32)
    wpack = np.zeros((P, NW), np.float32)
    wpack[:F_NODE, W1_OFF:W1_OFF + H] = W1
    wpack[F_NODE:, W1_OFF + H:W1_OFF + 2 * H] = W1
    for i in range(P // H):
        wpack[H * i:H * i + H, W2_OFF + H * i:W2_OFF + H * i + H] = W2
    wpack[:, B1_OFF] = np.tile(np.asarray(b1, np.float32), P // H)
    wpack[:, B2_OFF] = np.tile(np.asarray(b2, np.float32), P // H)
    wpack[:F_COL, CW1_OFF:CW1_OFF + H] = np.asarray(col_W1, np.float32)
    wpack[:H, CB1_OFF] = np.asarray(col_b1, np.float32)

    in_maps = []
    for c in range(N_CORES):
        n0 = c * NODES_PER_CORE
        half = NODES_PER_CORE // 2
        xa = x[n0:n0 + half].T                      # [64, 12500] view
        xb = x[n0 + half:n0 + NODES_PER_CORE].T
        xT = np.ascontiguousarray(
            np.concatenate([xa, xb], axis=0), dtype=np.float32).astype(NPDT)
        wp = wpack.copy()
        wp[:F_COL, COLT_OFF:COLT_OFF + COLN] = colf[c * COLN:(c + 1) * COLN].T
        in_maps.append({"xT": xT, "wpack": wp.astype(NPDT)})
    return in_maps


def kernel(node_features, col_features, edge_index, W1, b1, W2, b2,
           node_fc_W, node_fc_b, col_W1, col_b1, col_W2, col_b2,
           fc_W, fc_b, out_W, out_b):
    global LAST_EXEC_TIME_NS, LAST_RESULTS
    # edge_index provably does not affect the output (see module docstring).
    in_maps = _prep_in_maps(node_features, col_features,
                            W1, b1, W2, b2, col_W1, col_b1)
    nc = _get_nc(relu1_on_dve=not np.any(np.asarray(b1)))
    res = run_bass_kernel_spmd(nc, in_maps, core_ids=list(range(N_CORES)),
                               trace=PROFILE)
    LAST_EXEC_TIME_NS = res.exec_time_ns
    LAST_RESULTS = res
    outs = res.results

    node_fc_W = np.asarray(node_fc_W, np.float32)
    col_W2 = np.asarray(col_W2, np.float32)
    node_avg = np.zeros((B, 1), np.float32)
    col_avg = np.zeros((B, 1), np.float32)
    for b in range(B):
        ns = (outs[2 * b]["node_acc"].reshape(P // H, H).sum(axis=0) +
              outs[2 * b + 1]["node_acc"].reshape(P // H, H).sum(axis=0))
        cs = (outs[2 * b]["col_acc"].reshape(H) +
              outs[2 * b + 1]["col_acc"].reshape(H))
        node_avg[b, 0] = (ns / np.float32(N)) @ node_fc_W[:, 0] + \
            np.asarray(node_fc_b, np.float32)[0]
        col_avg[b, 0] = (cs / np.float32(C)) @ col_W2[:, 0] + \
            np.asarray(col_b2, np.float32)[0]

    combined = np.concatenate([node_avg, col_avg], axis=1)      # [B, 2]
    z = np.maximum(combined @ np.asarray(fc_W, np.float32) +
                   np.asarray(fc_b, np.float32), 0.0)
    out = z @ np.asarray(out_W, np.float32) + np.asarray(out_b, np.float32)
    return out.astype(np.float32)


# revision 42
# speedup vs baseline: 1.5151x; 1.0538x over previous
"""Trainium2 Bass kernel for nn_CriticNetwork (gnn_message_passing).

Key mathematical simplification (verified numerically against the
reference): the reference broadcasts edge_index to (B, 2, E) and
reshapes to (2, B*E).  Row-major reshape interleaves the src/dst
blocks so the resulting src and dst arrays are ELEMENTWISE EQUAL --
every edge is a self-edge v->v.  With GCN normalization
(deg = 1 + 2*count(v), each self-edge contributes x[v]/deg, plus the
explicit self-loop) the aggregate is exactly deg * x[v]/deg = x[v].
Both GCNConv layers therefore collapse to plain linear layers:

    x = relu(x @ W1 + b1); x = relu(x @ W2 + b2)
    node_avg[b] = mean_n(x[b, n] @ node_fc_W) + node_fc_b
    col path is a plain 2-layer MLP; final head is a tiny [4,2] MLP.

Since node_fc / col_W2 are applied linearly after the last relu, the
device only needs per-(batch-slice) SUMS of the hidden activations:
each core processes 25000 nodes (half a batch) + 500 col rows and
returns two small accumulator vectors; the host applies the final
(tiny) linear head.

Device layout per core:
  xT_packed [128, 12500]: rows 0-63  = 64 features of nodes [0, 12500)
                          rows 64-127 = 64 features of nodes [12500, 25000)
  L1 matmul: lhsT = blockdiag(W1, W1) [128, 32] -> h1.T bands [32, 512]
  4 L1 matmuls stack bands in one PSUM bank -> [128, 512]
  relu (ScalarE, bias fused) -> SBUF
  L2 matmul: lhsT = blockdiag(W2 x8) [128, 128] -> [128, 512] PSUM
  relu + accumulate (ScalarE accum_out = per-partition row sum)
  final: reduce accum columns -> node_acc [128, 1] (8 bands of 16)

All constants (weights, biases, col features) ship in ONE packed DMA
("wpack") and a zero-valued warmup matmul consumes it first: the PE
LDWEIGHTS instruction can carry only ONE semaphore wait, so every real
matmul must depend on at most one un-synced DMA lane (its x chunk).
"""

import ml_dtypes
import numpy as np

import concourse.bacc as bacc
import concourse.bass as bass
import concourse.mybir as mybir
import concourse.tile as tile
from concourse.bass_utils import run_bass_kernel_spmd

P = 128
N_CORES = 8
B, N, F_NODE, H = 4, 50000, 64, 16
NODES_PER_CORE = (B * N) // N_CORES        # 25000
COLS = NODES_PER_CORE // 2                 # 12500 packed columns (2 nodes/col)
MM = 512                                   # fp32 matmul max moving free dim
SUPER = 4 * MM                             # 2048 columns per PSUM-bank group
N_CHUNKS = (COLS + SUPER - 1) // SUPER     # 7 (6 full + 212-col tail)
C, F_COL = 1000, 32
COLN = (B * C) // N_CORES                  # 500 col rows per core

# wpack column layout
W1_OFF = 0                                  # [128, 32] blockdiag(W1, W1)
W2_OFF = W1_OFF + 2 * H                     # [128, 128] blockdiag(W2 x8)
B1_OFF = W2_OFF + P                         # [128, 1] b1 tiled x8
B2_OFF = B1_OFF + 1                         # [128, 1] b2 tiled x8
CW1_OFF = B2_OFF + 1                        # [32, 16] col_W1 (rows 0-31)
CB1_OFF = CW1_OFF + H                       # [16, 1] col_b1 (rows 0-15)
ZPAD_OFF = CB1_OFF + 1                      # [128, 1] zeros (warmup operand)
COLT_OFF = ZPAD_OFF + 1                     # [32, 500] colT (rows 0-31)
NW = COLT_OFF + COLN                        # 680

DT = mybir.dt.bfloat16                     # matmul-operand dtype on device
NPDT = ml_dtypes.bfloat16

PROFILE = False        # set True (e.g. from test.py) to collect NTFF timing
CHECK_WAITS = True     # build-time guard: one semaphore wait per compute inst
LAST_EXEC_TIME_NS = None
LAST_RESULTS = None

_NC_CACHE = {}


def _build_nc(relu1_on_dve=True):
    f32 = mybir.dt.float32
    Relu = mybir.ActivationFunctionType.Relu
    # Bacc (not raw Bass): its finalize() runs move_matmul_waits_to_-
    # ldweights + generate_event_semaphores, which legalize schedules
    # against the TRN2 one-semaphore-wait-per-instruction limit.
    nc = bacc.Bacc("TRN2")

    xT = nc.dram_tensor("xT", [P, COLS], DT, kind="ExternalInput")
    wpack = nc.dram_tensor("wpack", [P, NW], DT, kind="ExternalInput")
    node_acc = nc.dram_tensor("node_acc", [P, 1], f32, kind="ExternalOutput")
    col_acc = nc.dram_tensor("col_acc", [H, 1], f32, kind="ExternalOutput")

    with tile.TileContext(nc) as tc:
        with (
            tc.tile_pool(name="consts", bufs=1) as consts,
            tc.tile_pool(name="xin", bufs=4) as xin,
            tc.tile_pool(name="work", bufs=2) as work,
            tc.tile_pool(name="outp", bufs=1) as outp,
            tc.tile_pool(name="psum", bufs=1, space="PSUM") as psum,
        ):
            wp = consts.tile([P, NW], DT)
            nc.sync.dma_start(wp[:], wpack[:])
            w1_t = wp[:, W1_OFF:W1_OFF + 2 * H]
            w2_t = wp[:, W2_OFF:W2_OFF + P]
            b1_t = wp[:, B1_OFF:B1_OFF + 1]
            b2_t = wp[:, B2_OFF:B2_OFF + 1]
            cw1_t = wp[:F_COL, CW1_OFF:CW1_OFF + H]
            cb1_t = wp[:H, CB1_OFF:CB1_OFF + 1]
            zc_t = wp[:, ZPAD_OFF:ZPAD_OFF + 1]
            colT_t = wp[:F_COL, COLT_OFF:COLT_OFF + COLN]

            # Zero stats ON the engine that will accumulate into it (same-
            # engine WAW needs no cross-engine wait).  Reading wpack here
            # also syncs that engine with the wpack DMA lane up front.
            # zeros path: everything post-PE lives on DVE and the Scalar
            # engine is left completely idle (no ACT_TABLE_LOAD either).
            stats = outp.tile([P, N_CHUNKS + 1], f32)
            if relu1_on_dve:
                nc.vector.tensor_scalar_mul(stats[:], wp[:, :N_CHUNKS + 1], 0.0)
            else:
                nc.scalar.mul(stats[:], wp[:, :N_CHUNKS + 1], 0.0)

            # Persistent PSUM tiles (allocated once, manually alternated):
            # a per-chunk pool tile would get a slot-recycle writer guard,
            # an extra PE-sem wait on the first matmul of each chunk -- and
            # the PE LDWEIGHTS instruction can carry only ONE wait.
            NBUF = 3
            ps1_t = [psum.tile([P, MM], f32, tag=f"ps1_{k}", name=f"ps1_{k}")
                     for k in range(NBUF)]
            ps2_t = [psum.tile([P, MM], f32, tag=f"ps2_{k}", name=f"ps2_{k}")
                     for k in range(NBUF)]
            h1r_t = [work.tile([P, MM], DT, tag=f"h1r_{k}", name=f"h1r_{k}")
                     for k in range(NBUF)]
            scr_t = [work.tile([P, MM], DT, tag=f"scr_{k}", name=f"scr_{k}")
                     for k in range(NBUF)]

            # Warmup matmul: syncs PE with the wpack DMA using a single
            # wait, so every later matmul has the wpack lane subsumed.
            # Reads the zero pad column -> contributes exactly 0.0 to
            # stats' spare column (kept live through that write).
            psd = psum.tile([1, 1], f32, tag="psd")
            nc.tensor.matmul(psd[0:1, 0:1], zc_t, zc_t, start=True, stop=True)
            if relu1_on_dve:
                nc.vector.tensor_copy(stats[0:1, N_CHUNKS:N_CHUNKS + 1],
                                      psd[0:1, 0:1])
            else:
                nc.scalar.copy(stats[0:1, N_CHUNKS:N_CHUNKS + 1], psd[0:1, 0:1])

            for s in range(N_CHUNKS):
                c0 = s * SUPER
                cols = min(SUPER, COLS - c0)
                nb = (cols + MM - 1) // MM
                act_w = cols if nb == 1 else cols // nb
                assert act_w * nb == cols, (s, cols, nb)

                x_t = xin.tile([P, SUPER], DT, tag="x")
                nc.sync.dma_start(x_t[:, :cols], xT[:, c0:c0 + cols])

                ps1 = ps1_t[s % NBUF]
                for bnd in range(nb):
                    w = min(MM, cols - bnd * MM)
                    nc.tensor.matmul(
                        ps1[32 * bnd:32 * bnd + 32, :w],
                        w1_t,
                        x_t[:, bnd * MM:bnd * MM + w],
                        start=True, stop=True,
                        tile_position=(0, 32 * bnd),
                    )
                used = 32 * nb

                h1r = h1r_t[s % NBUF]
                if relu1_on_dve:
                    # b1 is structurally zero (setup_inputs uses
                    # jnp.zeros), so relu1 is a plain max with an
                    # immediate -- keeps DVE free of a wpack-DMA wait.
                    nc.vector.tensor_scalar_max(
                        h1r[:used, :act_w], ps1[:used, :act_w], 0.0)
                else:
                    nc.scalar.activation(
                        h1r[:used, :act_w], ps1[:used, :act_w], Relu,
                        bias=b1_t[:used, :],
                    )

                ps2 = ps2_t[s % NBUF]
                nc.tensor.matmul(
                    ps2[:used, :act_w],
                    w2_t[:used, :used],
                    h1r[:used, :act_w],
                    start=True, stop=True,
                )
                scr = scr_t[s % NBUF]
                if relu1_on_dve:
                    # b2 structurally zero: relu2 + row-sum in one DVE op.
                    nc.vector.tensor_scalar(
                        scr[:used, :act_w], ps2[:used, :act_w], 0.0, 0.0,
                        mybir.AluOpType.max, mybir.AluOpType.add,
                        accum_out=stats[:used, s:s + 1],
                    )
                else:
                    nc.scalar.activation(
                        scr[:used, :act_w], ps2[:used, :act_w], Relu,
                        bias=b2_t[:used, :],
                        accum_out=stats[:used, s:s + 1],
                    )

            # column-features path (tiny): h = relu(col @ col_W1 + col_b1)
            psc = psum.tile([H, COLN], f32, tag="psc")
            nc.tensor.matmul(psc[:, :], cw1_t, colT_t, start=True, stop=True)
            colscr = outp.tile([H, COLN], f32)
            col_sb = outp.tile([H, 1], f32)
            if relu1_on_dve:
                # col_b1 structurally zero as well.
                nc.vector.tensor_scalar(
                    colscr[:], psc[:], 0.0, 0.0,
                    mybir.AluOpType.max, mybir.AluOpType.add,
                    accum_out=col_sb[:])
            else:
                nc.scalar.activation(colscr[:], psc[:], Relu,
                                     bias=cb1_t, accum_out=col_sb[:])

            node_sb = outp.tile([P, 1], f32)
            nc.vector.tensor_reduce(node_sb[:], stats[:],
                                    axis=mybir.AxisListType.X,
                                    op=mybir.AluOpType.add)
            nc.sync.dma_start(node_acc[:], node_sb[:])
            nc.sync.dma_start(col_acc[:], col_sb[:])

    nc.finalize()

    # Verify the legalization: at most one wait per instruction
    # (InstEventSemaphore may carry two).
    if CHECK_WAITS:
        for blk in nc.m.functions[0].blocks:
            for inst in blk.instructions:
                si = inst.sync_info
                nwait = len(si.on_wait) if si and si.on_wait else 0
                limit = 2 if type(inst).__name__ in (
                    "InstEventSemaphore", "InstDrain", "InstDMACopy") else 1
                assert nwait <= limit, (
                    inst.name, type(inst).__name__,
                    [w.ant_name for w in si.on_wait])
    return nc


def _get_nc(relu1_on_dve=True):
    key = ("nc", relu1_on_dve)
    if key not in _NC_CACHE:
        _NC_CACHE[key] = _build_nc(relu1_on_dve)
    return _NC_CACHE[key]


def _prep_in_maps(node_features, col_features, W1, b1, W2, b2, col_W1, col_b1):
    x = np.ascontiguousarray(node_features, dtype=np.float32).reshape(B * N, F_NODE)
    colf = np.ascontiguousarray(col_features, dtype=np.float32).reshape(B * C, F_COL)

    W1 = np.asarray(W1, np.float32)
    W2 = np.asarray(W2, np.float32)
    wpack = np.zeros((P, NW), np.float32)
    wpack[:F_NODE, W1_OFF:W1_OFF + H] = W1
    wpack[F_NODE:, W1_OFF + H:W1_OFF + 2 * H] = W1
    for i in range(P // H):
        wpack[H * i:H * i + H, W2_OFF + H * i:W2_OFF + H * i + H] = W2
    wpack[:, B1_OFF] = np.tile(np.asarray(b1, np.float32), P // H)
    wpack[:, B2_OFF] = np.tile(np.asarray(b2, np.float32), P // H)
    wpack[:F_COL, CW1_OFF:CW1_OFF + H] = np.asarray(col_W1, np.float32)
    wpack[:H, CB1_OFF] = np.asarray(col_b1, np.float32)

    in_maps = []
    for c in range(N_CORES):
        n0 = c * NODES_PER_CORE
        half = NODES_PER_CORE // 2
        xa = x[n0:n0 + half].T                      # [64, 12500] view
        xb = x[n0 + half:n0 + NODES_PER_CORE].T
        xT = np.ascontiguousarray(
            np.concatenate([xa, xb], axis=0), dtype=np.float32).astype(NPDT)
        wp = wpack.copy()
        wp[:F_COL, COLT_OFF:COLT_OFF + COLN] = colf[c * COLN:(c + 1) * COLN].T
        in_maps.append({"xT": xT, "wpack": wp.astype(NPDT)})
    return in_maps


def kernel(node_features, col_features, edge_index, W1, b1, W2, b2,
           node_fc_W, node_fc_b, col_W1, col_b1, col_W2, col_b2,
           fc_W, fc_b, out_W, out_b):
    global LAST_EXEC_TIME_NS, LAST_RESULTS
    # edge_index provably does not affect the output (see module docstring).
    in_maps = _prep_in_maps(node_features, col_features,
                            W1, b1, W2, b2, col_W1, col_b1)
    zeros_path = not (np.any(np.asarray(b1)) or np.any(np.asarray(b2))
                      or np.any(np.asarray(col_b1)))
    nc = _get_nc(relu1_on_dve=zeros_path)
    res = run_bass_kernel_spmd(nc, in_maps, core_ids=list(range(N_CORES)),
                               trace=PROFILE)
    LAST_EXEC_TIME_NS = res.exec_time_ns
    LAST_RESULTS = res
    outs = res.results

    node_fc_W = np.asarray(node_fc_W, np.float32)
    col_W2 = np.asarray(col_W2, np.float32)
    node_avg = np.zeros((B, 1), np.float32)
    col_avg = np.zeros((B, 1), np.float32)
    for b in range(B):
        ns = (outs[2 * b]["node_acc"].reshape(P // H, H).sum(axis=0) +
              outs[2 * b + 1]["node_acc"].reshape(P // H, H).sum(axis=0))
        cs = (outs[2 * b]["col_acc"].reshape(H) +
              outs[2 * b + 1]["col_acc"].reshape(H))
        node_avg[b, 0] = (ns / np.float32(N)) @ node_fc_W[:, 0] + \
            np.asarray(node_fc_b, np.float32)[0]
        col_avg[b, 0] = (cs / np.float32(C)) @ col_W2[:, 0] + \
            np.asarray(col_b2, np.float32)[0]

    combined = np.concatenate([node_avg, col_avg], axis=1)      # [B, 2]
    z = np.maximum(combined @ np.asarray(fc_W, np.float32) +
                   np.asarray(fc_b, np.float32), 0.0)
    out = z @ np.asarray(out_W, np.float32) + np.asarray(out_b, np.float32)
    return out.astype(np.float32)
